# revision 1
# baseline (speedup 1.0000x reference)
"""Trainium2 Bass kernel for nn_Generator_34127810134219 (gnn_message_passing).

Strategy
--------
The reference relmod builds a [B,N,N] score matrix S = c*x@x^T (diag masked)
and computes wr*(S@U)/N + x.  Algebraically (verified to 4e-7 rel err):

    S@U = c*( x @ (x^T U) - ||x_i||^2 * U_i )

which collapses O(B*N^2*D) work into O(B*N*D^2).  The whole network is then a
memory-light pointwise/matmul pipeline over B*N = 32768 tokens with feature
dims <= 12.

Sharding: data-parallel over batch, 2 batches per core (8 cores).  The only
cross-core coupling is BatchNorm statistics (mean/var per n over batch and
feature dims) - exchanged as tiny [8,512] partial-sum tiles via AllGather
(3x), then reduced locally.  relmod is fully batch-local.

On-chip layout: feature-major, group-packed.  Per core 4096 tokens are split
into 8 groups of 512; group g lives on partitions [16g, 16g+C).  All fc
layers become single 128x512 matmuls with block-diagonal weights (float32r
for full-rate PE).  The per-batch Gram matrix G = x^T U is built with PE
transposes + matmuls; per-group partials are folded per batch as
mask . (Phi^T P_masked Phi) . mask with a fold matrix Phi - PE matmuls only,
no cross-partition vector ops.

All constant patterns (selectors, masks, Phi, block-diagonal weights) are
built on-chip from affine_select primitives + PE matmuls: DMA triggers are
the dominant fixed cost (~0.6us each on the shared HWDGE) so the kernel
issues only ~40 of them.
"""

import numpy as np

import concourse.bass as bass
import concourse.bacc as bacc
import concourse.tile as tile
import concourse.mybir as mybir
from concourse.bass_utils import run_bass_kernel_spmd
from concourse.masks import make_identity

FP32 = mybir.dt.float32
F32R = mybir.dt.float32r
AF = mybir.ActivationFunctionType
OP = mybir.AluOpType

B, N, F = 16, 2048, 3
D2, D4 = 6, 12
NCORES = 8
BPC = B // NCORES          # batches per core
T = BPC * N                # tokens per core
NG = 8                     # groups per core
L = T // NG                # free-dim length (512)
GS = 16                    # partition stride per group
EPS = 1e-5
SAFE_RSTD = False     # Ln+Exp instead of Abs_reciprocal_sqrt
SAFE_SIGMOID = True   # ACT Sigmoid instead of exp+reciprocal_approx

# (name, shape) of every external input except x
PARAM_SPECS = [
    ("fc1_w", (D2, F)), ("fc1_b", (D2,)), ("bn1_g", (N,)), ("bn1_b", (N,)),
    ("fc2_w", (D4, D2)), ("fc2_b", (D4,)), ("bn2_g", (N,)), ("bn2_b", (N,)),
    ("fc3_w", (D4, D4)), ("fc3_b", (D4,)),
    ("u1_w", (D4, D4)), ("u1_b", (D4,)), ("ps1", (1,)), ("ph1", (1,)), ("wr1", (1,)),
    ("u2_w", (D4, D4)), ("u2_b", (D4,)), ("ps2", (1,)), ("ph2", (1,)), ("wr2", (1,)),
    ("u3_w", (D4, D4)), ("u3_b", (D4,)), ("ps3", (1,)), ("ph3", (1,)), ("wr3", (1,)),
    ("u4_w", (D4, D4)), ("u4_b", (D4,)), ("ps4", (1,)), ("ph4", (1,)), ("wr4", (1,)),
    ("fc4_w", (D2, D4)), ("fc4_b", (D2,)), ("bn4_g", (N,)), ("bn4_b", (N,)),
    ("fc5_w", (F, D2)), ("fc5_b", (F,)),
    ("fc6_w", (1, F)), ("fc6_b", (1,)), ("fc7_w", (2, F)), ("fc7_b", (2,)),
]

# weight slot order inside the compact tile (each slot is 16 cols)
W_ORDER = ["fc1_w", "fc2_w", "fc3_w", "u1_w", "u2_w", "u3_w", "u4_w",
           "fc4_w", "fc5_w", "fc67_w"]
B_ORDER = ["fc1_b", "fc2_b", "fc3_b", "u1_b", "u2_b", "u3_b", "u4_b",
           "fc4_b", "fc5_b", "fc67_b"]


def _build(single_core=False):
    nc = bacc.Bacc(
        "TRN2",
        target_bir_lowering=False,
        debug=False,
        enable_asserts=False,
        num_devices=1 if single_core else NCORES,
    )

    x_d = nc.dram_tensor("x", [BPC, N, F], FP32, kind="ExternalInput")
    prm = {
        name: nc.dram_tensor(name, list(shape), FP32, kind="ExternalInput")
        for name, shape in PARAM_SPECS
    }
    out_d = nc.dram_tensor("out", [BPC, N, F], FP32, kind="ExternalOutput")

    with tile.TileContext(nc) as tc:
        with (
            tc.tile_pool(name="consts", bufs=1) as cp,
            tc.tile_pool(name="sb", bufs=1) as sb,
            tc.tile_pool(name="pp", bufs=1, space="PSUM") as pp,
            tc.tile_pool(name="dram", bufs=1, space="DRAM") as dr,
        ):
            _emit(nc, tc, cp, sb, pp, dr, x_d, prm, out_d,
                  single_core=single_core)

    nc.compile()
    return nc


def _emit(nc, tc, cp, sb, pp, dr, x_d, prm, out_d, single_core=False):
    def mmr(out, lhsT, rhs, **kw):
        """float32r matmul: full-rate PE for fp32 bits (reduced mult precision)."""
        nc.tensor.matmul(out, lhsT.bitcast(F32R), rhs.bitcast(F32R), **kw)

    def r(ap):
        """f32r view for producer outputs feeding f32r matmuls (rounds)."""
        return ap.bitcast(F32R)

    eps_t = cp.tile([128, 1], FP32, name="eps_t")
    nc.gpsimd.memset(eps_t[:], EPS)
    # first ACT instruction uses Ln so walrus resolves the
    # natural_log_exp_and_others table set once for the whole kernel
    actwarm = sb.tile([1, 1], FP32, name="actwarm")
    nc.scalar.activation(actwarm[:], eps_t[0:1, :],
                         AF.Ln if SAFE_RSTD else AF.Abs_reciprocal_sqrt)

    # ================= affine-built base selectors (Pool engine) =============
    def affine_sel(t, pattern, cm):
        """t := 1.0 where cm*p + pattern.idx == 0 else 0."""
        nc.gpsimd.memset(t, 0.0)
        nc.gpsimd.affine_select(
            out=t, in_=t, compare_op=OP.not_equal, fill=1.0,
            base=0, pattern=pattern, channel_multiplier=cm)

    # input load first so the network isn't gated on init DMAs
    X = sb.tile([128, L], FP32, name="X")
    nc.gpsimd.memset(X[:], 0.0)
    for g in range(NG):
        b, n0 = g // 4, (g % 4) * L
        eng = nc.sync
        eng.dma_start(X[GS * g:GS * g + F, :],
                      x_d[b, n0:n0 + L, :].rearrange("n c -> c n"))

    # bc8[g, (g',c)] = [g'==g]
    bc8 = cp.tile([NG, 128], FP32, name="bc8")
    affine_sel(bc8[:].rearrange("p (g c) -> p g c", c=GS), [[1, NG], [0, GS]], -1)
    # bc4[j, (g,c)] = [g%4==j]
    bc4 = cp.tile([4, 128], FP32, name="bc4")
    affine_sel(bc4[:].rearrange("p (h j c) -> p h j c", j=4, c=GS),
               [[0, 2], [1, 4], [0, GS]], -1)
    # bcB[b, (g,c)] = [g//4==b]
    bcB = cp.tile([2, 128], FP32, name="bcB")
    affine_sel(bcB[:].rearrange("p (b j c) -> p b j c", j=4, c=GS),
               [[1, 2], [0, 4], [0, GS]], -1)
    # RepSel12[ci', (g,ci)] = [ci==ci'] (ci'<12)
    rsel12 = cp.tile([D4, 128], FP32, name="rsel12")
    affine_sel(rsel12[:].rearrange("p (g c) -> p g c", c=GS), [[0, NG], [1, GS]], -1)
    # RepSel16
    rsel16 = cp.tile([GS, 128], FP32, name="rsel16")
    affine_sel(rsel16[:].rearrange("p (g c) -> p g c", c=GS), [[0, NG], [1, GS]], -1)
    # S8[j, (r,j')] = [j'==j]  (for rank-reduction tiles)
    s8 = cp.tile([8, 64], FP32, name="s8")
    affine_sel(s8[:].rearrange("p (r j) -> p r j", j=8), [[0, 8], [1, 8]], -1)

    ident128 = cp.tile([128, 128], FP32, name="ident128")
    make_identity(nc, ident128[:])
    ones12 = cp.tile([D4, 1], FP32, name="ones12")
    nc.gpsimd.memset(ones12[:], 1.0)
    ones1 = cp.tile([1, 128], FP32, name="ones1")
    nc.gpsimd.memset(ones1[:], 1.0)
    # ================= PE-derived constant tiles =============================
    # mask_diag[(g,c),(g',c')] = [g==g']
    mask_ps = pp.tile([128, 128], FP32, name="mask_ps", tag="b0", padded_shape=[128, L])
    nc.tensor.matmul(mask_ps[:], bc8[:], bc8[:])
    mask_diag = cp.tile([128, 128], FP32, name="mask_diag")
    nc.scalar.activation(mask_diag[:], mask_ps[:], AF.Copy)
    # onesfold [128,4] = bc4^T (needed by the first bn_send pack matmuls)
    of_ps = pp.tile([128, 4], FP32, name="of_ps", tag="b3", padded_shape=[128, L])
    nc.tensor.transpose(of_ps[:], bc4[:], ident128[0:4, 0:4])
    onesfold = cp.tile([128, 4], FP32, name="onesfold")
    nc.scalar.activation(r(onesfold[:]), of_ps[:], AF.Copy)
    # deferred consts (phi/ones_c16/colmask12) are emitted in the bn1
    # AllGather window so they don't sit ahead of fc1 in the PE queue
    phi = cp.tile([128, 128], FP32, name="phi")
    ones_c16 = cp.tile([128, NG], FP32, name="ones_c16")
    colmask12 = cp.tile([128, 1], FP32, name="colmask12")

    def build_deferred_consts():
        crep_ps = pp.tile([128, 128], FP32, name="crep_ps", tag="b1",
                          padded_shape=[128, L])
        nc.tensor.matmul(crep_ps[:], rsel16[:], rsel16[:])
        crep = sb.tile([128, 128], FP32, name="crep")
        nc.scalar.activation(crep[:], crep_ps[:], AF.Copy)
        bmask_ps = pp.tile([128, 128], FP32, name="bmask_ps", tag="b2",
                           padded_shape=[128, L])
        nc.tensor.matmul(bmask_ps[:], bcB[:], bcB[:])
        nc.vector.tensor_tensor(r(phi[:]), bmask_ps[:], crep[:], OP.mult)
        oc_ps = pp.tile([128, NG], FP32, name="oc_ps", tag="b4",
                        padded_shape=[128, L])
        nc.tensor.transpose(oc_ps[:], bc8[:], ident128[0:NG, 0:NG])
        nc.scalar.activation(r(ones_c16[:]), oc_ps[:], AF.Copy)
        cm_ps = pp.tile([128, 1], FP32, name="cm_ps", tag="b5",
                        padded_shape=[128, L])
        nc.tensor.matmul(cm_ps[:], rsel12[:], ones12[:])
        nc.scalar.activation(colmask12[:], cm_ps[:], AF.Copy)
    # f32r-rounded copies of bc4/bc8 (mmr operands must have f32r producers)
    bc4r = cp.tile([4, 128], FP32, name="bc4r")
    nc.vector.tensor_copy(r(bc4r[:]), bc4[:])
    bc8r = cp.tile([NG, 128], FP32, name="bc8r")
    nc.vector.tensor_copy(r(bc8r[:]), bc8[:])
    # rank-reduction tiles [64,8] = S8^T scaled by 1/count
    rr_ps = pp.tile([64, 8], FP32, name="rr_ps", tag="b6", padded_shape=[128, L])
    nc.tensor.transpose(rr_ps[:], s8[:], ident128[0:8, 0:8])
    rr96 = cp.tile([64, 8], FP32, name="rr96")
    nc.scalar.activation(r(rr96[:]), rr_ps[:], AF.Copy, scale=1.0 / 96.0)
    rr192 = cp.tile([64, 8], FP32, name="rr192")
    nc.scalar.activation(r(rr192[:]), rr_ps[:], AF.Copy, scale=1.0 / 192.0)

    # ================= weights / biases ======================================
    # per-weight: Wc[ci,co] -DMA-> [12,16] tile; tp = Wc^T.rsel12 gives the
    # partition-replicated transpose; sp = tp^T.rsel16 spreads along free;
    # mask leaves the block-diagonal lhsT.  build_weight() is emitted at
    # chosen points so init work hides inside collective-wait windows
    # (engines run their streams in order).
    WBD = {}
    _WC = {}

    def load_wc(wname):
        wc = cp.tile([D4, GS], FP32, name=f"wc_{wname}")
        nc.vector.memset(wc[:], 0.0)
        if wname == "fc67_w":
            nc.gpsimd.dma_start(wc[0:F, 0:1],
                                prm["fc6_w"][:, :].rearrange("o i -> i o"))
            nc.gpsimd.dma_start(wc[0:F, 1:3],
                                prm["fc7_w"][:, :].rearrange("o i -> i o"))
        else:
            o, i = prm[wname].shape
            nc.gpsimd.dma_start(wc[0:i, 0:o],
                                prm[wname][:, :].rearrange("o i -> i o"))
        _WC[wname] = wc

    def finish_weight(wname):
        wc = _WC[wname]
        tp = pp.tile([GS, 128], FP32, name=f"wt_{wname}", tag="b6",
                     padded_shape=[128, L])
        nc.tensor.matmul(tp[:], wc[:], rsel12[:])
        ts = sb.tile([GS, 128], FP32, name=f"ws_{wname}", tag="wts")
        nc.scalar.activation(ts[:], tp[:], AF.Copy)
        sp = pp.tile([128, 128], FP32, name=f"wsp_{wname}", tag="b7",
                     padded_shape=[128, L])
        nc.tensor.matmul(sp[:], ts[:], rsel16[:])
        wt = cp.tile([128, 128], FP32, name=f"W_{wname}")
        nc.vector.tensor_tensor(r(wt[:]), sp[:], mask_diag[:], OP.mult)
        WBD[wname] = wt

    load_wc("fc1_w")
    finish_weight("fc1_w")

    BIAS = {}
    _BCV = {}

    def load_bcv(bname):
        bcv = cp.tile([D4, 1], FP32, name=f"bcv_{bname}")
        nc.vector.memset(bcv[:], 0.0)
        if bname == "fc67_b":
            nc.gpsimd.dma_start(bcv[0:1, 0:1],
                                prm["fc6_b"][:].rearrange("(o u) -> o u", u=1))
            nc.gpsimd.dma_start(bcv[1:3, 0:1],
                                prm["fc7_b"][:].rearrange("(o u) -> o u", u=1))
        else:
            cnt = prm[bname].shape[0]
            nc.gpsimd.dma_start(bcv[0:cnt, 0:1],
                                prm[bname][:].rearrange("(o u) -> o u", u=1))
        _BCV[bname] = bcv

    def finish_bias(bname):
        bps = pp.tile([128, 1], FP32, name=f"bps_{bname}", tag="b2",
                      padded_shape=[128, L])
        nc.tensor.matmul(bps[:], rsel12[:], _BCV[bname][:])
        bt = cp.tile([128, 1], FP32, name=f"bias_{bname}")
        nc.scalar.activation(bt[:], bps[:], AF.Copy)
        BIAS[bname] = bt

    load_bcv("fc1_b")
    finish_bias("fc1_b")

    # bn scale/shift as [4, 512]: row j covers n in [512j, 512j+512)
    def bn_vec(name):
        t = cp.tile([4, L], FP32, name=f"v_{name}")
        nc.gpsimd.dma_start(t[:], prm[name][:].rearrange("(j t) -> j t", t=L))
        return t

    bng, bnb = {}, {}

    def load_bn_vecs(k):
        bng[k] = bn_vec(f"{k}_g")
        bnb[k] = bn_vec(f"{k}_b")

    load_bn_vecs("bn1")
    bnb_bc = {}

    def build_bnb_bc(k):
        bps = pp.tile([128, L], FP32, name=f"bnbps_{k}", tag="b3")
        nc.tensor.matmul(bps[:], bc4[:], bnb[k][:])
        bsb = cp.tile([128, L], FP32, name=f"bnbbc_{k}")
        nc.scalar.activation(bsb[:], bps[:], AF.Copy)
        bnb_bc[k] = bsb

    build_bnb_bc("bn1")

    # relmod scale a_r = wr*ps*ph/N as [128,1]
    a_r = []

    def emit_relmod_scales():
      for i in (1, 2, 3, 4):
        pst = sb.tile([1, 1], FP32, name=f"ps_{i}", tag="sc1")
        pht = sb.tile([1, 1], FP32, name=f"ph_{i}", tag="sc2")
        wrt = sb.tile([1, 1], FP32, name=f"wr_{i}", tag="sc3")
        nc.gpsimd.dma_start(pst[:], prm[f"ps{i}"][:].rearrange("(o u) -> o u", u=1))
        nc.gpsimd.dma_start(pht[:], prm[f"ph{i}"][:].rearrange("(o u) -> o u", u=1))
        nc.gpsimd.dma_start(wrt[:], prm[f"wr{i}"][:].rearrange("(o u) -> o u", u=1))
        nc.vector.tensor_tensor(pst[:], pst[:], pht[:], OP.mult)
        nc.vector.tensor_tensor(pst[:], pst[:], wrt[:], OP.mult)
        nc.vector.tensor_scalar_mul(pst[:], pst[:], 1.0 / N)
        pb = pp.tile([128, 1], FP32, name=f"psc_{i}", tag="b3",
                     padded_shape=[128, L])
        nc.tensor.matmul(pb[:], ones1[:], pst[:])
        at = cp.tile([128, 1], FP32, name=f"a_r{i}")
        nc.scalar.activation(at[:], pb[:], AF.Copy)
        a_r.append(at)

    # ================= helpers ===============================================
    def fc(w, src, name, plain=False):
        ps = pp.tile([128, L], FP32, name=f"psfc_{name}", tag="b0")
        if plain:
            nc.tensor.matmul(ps[:], w[:], src[:])
        else:
            mmr(ps[:], w[:], src[:])
        return ps

    def bn_send(h_ps, bias, tag):
        """fc PSUM -> biased hs + partial stats -> AllGather kickoff."""
        hs = sb.tile([128, L], FP32, name=f"hs_{tag}")
        nc.scalar.add(hs[:], h_ps[:], bias)
        sq = sb.tile([128, L], FP32, name=f"sq_{tag}")
        nc.scalar.activation(sq[:], h_ps[:], AF.Square, bias=bias)
        pk_s = pp.tile([4, L], FP32, name=f"pks_{tag}", tag="b1", padded_shape=[128, L])
        pk_q = pp.tile([4, L], FP32, name=f"pkq_{tag}", tag="b2", padded_shape=[128, L])
        nc.tensor.matmul(pk_s[:], onesfold[:], hs[:])
        nc.tensor.matmul(pk_q[:], onesfold[:], sq[:])
        sk_s = sb.tile([4, L], FP32, name=f"sks_{tag}")
        sk_q = sb.tile([4, L], FP32, name=f"skq_{tag}")
        nc.scalar.activation(sk_s[:], pk_s[:], AF.Copy)
        nc.vector.tensor_copy(sk_q[:], pk_q[:])
        cc_in = dr.tile([8, L], FP32, name=f"ccin_{tag}")
        cc_out = dr.tile([64, L], FP32, name=f"ccout_{tag}")
        nc.sync.dma_start(cc_in[0:4, :], sk_s[:])
        nc.scalar.dma_start(cc_in[4:8, :], sk_q[:])
        if single_core:
            # timing-only stand-in for the AllGather (TimelineSim path);
            # 4 serialized DMAs model the ~5us 8-core AllGather latency
            for r in range(4):
                nc.sync.dma_start(cc_out[8 * r:8 * r + 8, :], cc_in[:])
        else:
            nc.gpsimd.collective_compute(
                "AllGather",
                OP.bypass,
                replica_groups=[list(range(NCORES))],
                ins=[cc_in.opt()],
                outs=[cc_out.opt()],
            )
        return hs, cc_out

    def bn_recv(state, key, count_tile, tag):
        """Gathered stats -> bn(h) = a*(h-mean)+beta -> relu."""
        hs, cc_out = state
        gath = sb.tile([64, L], FP32, name=f"gath_{tag}")
        nc.sync.dma_start(gath[:], cc_out[:])
        m_ps = pp.tile([4, L], FP32, name=f"mps_{tag}", tag="b1", padded_shape=[128, L])
        q_ps = pp.tile([4, L], FP32, name=f"qps_{tag}", tag="b2", padded_shape=[128, L])
        nc.tensor.matmul(m_ps[:], count_tile[:, 0:4], gath[:])
        nc.tensor.matmul(q_ps[:], count_tile[:, 4:8], gath[:])
        mean = sb.tile([4, L], FP32, name=f"mean_{tag}")
        nc.scalar.activation(r(mean[:]), m_ps[:], AF.Copy)
        # h - mean (starts as soon as mean is up; off the rstd critical path)
        Mean_bc = pp.tile([128, L], FP32, name=f"Mbc_{tag}", tag="b4")
        mmr(Mean_bc[:], bc4r[:], mean[:])
        t1 = sb.tile([128, L], FP32, name=f"t1_{tag}")
        nc.vector.tensor_tensor(t1[:], hs[:], Mean_bc[:], OP.subtract)
        # a = gamma / sqrt(var+eps); Abs_reciprocal_sqrt is the one-op rstd
        # (ACT Rsqrt proper is banned; var+eps > 0 so abs is a no-op)
        msq = sb.tile([4, L], FP32, name=f"msq_{tag}")
        nc.scalar.activation(msq[:], m_ps[:], AF.Square)
        var = sb.tile([4, L], FP32, name=f"var_{tag}")
        nc.vector.tensor_tensor(var[:], q_ps[:], msq[:], OP.subtract)
        rstd = sb.tile([4, L], FP32, name=f"rstd_{tag}")
        if SAFE_RSTD:
            lv = sb.tile([4, L], FP32, name=f"lv_{tag}")
            nc.scalar.activation(lv[:], var[:], AF.Ln, bias=eps_t[0:4, :])
            nc.scalar.activation(rstd[:], lv[:], AF.Exp, scale=-0.5)
        else:
            nc.scalar.activation(rstd[:], var[:], AF.Abs_reciprocal_sqrt,
                                 bias=eps_t[0:4, :])
        a = sb.tile([4, L], FP32, name=f"a_{tag}")
        nc.vector.tensor_tensor(r(a[:]), rstd[:], bng[key][:], OP.mult)
        A_bc = pp.tile([128, L], FP32, name=f"Abc_{tag}", tag="b3")
        mmr(A_bc[:], bc4r[:], a[:])
        t2 = sb.tile([128, L], FP32, name=f"t2_{tag}")
        nc.vector.tensor_tensor(t2[:], t1[:], A_bc[:], OP.mult)
        t3 = sb.tile([128, L], FP32, name=f"t3_{tag}")
        nc.vector.tensor_tensor(t3[:], t2[:], bnb_bc[key][:], OP.add)
        hn = sb.tile([128, L], FP32, name=f"hn_{tag}")
        nc.vector.tensor_relu(r(hn[:]), t3[:])
        return hn

    def relmod(cur, wu, bu, at, idx):
        psU = pp.tile([128, L], FP32, name=f"psU_{idx}", tag="b0")
        mmr(psU[:], wu[:], cur[:])
        U = sb.tile([128, L], FP32, name=f"U_{idx}", tag="U")
        nc.scalar.activation(U[:], psU[:], AF.Relu, bias=bu)
        # s = sum_c cur^2 per token, broadcast to [128,L]
        sq = sb.tile([128, L], FP32, name=f"rsq_{idx}", tag="rsq")
        nc.scalar.activation(r(sq[:]), cur[:], AF.Square)
        psS = pp.tile([NG, L], FP32, name=f"psS_{idx}", tag="b5", padded_shape=[128, L])
        mmr(psS[:], ones_c16[:], sq[:])
        sS = sb.tile([NG, L], FP32, name=f"sS_{idx}", tag="sS")
        nc.vector.tensor_copy(r(sS[:]), psS[:])
        Sbc = pp.tile([128, L], FP32, name=f"Sbc_{idx}", tag="b3")
        mmr(Sbc[:], bc8r[:], sS[:])
        # transposes of cur and U (4x 128-chunks each)
        pTc = pp.tile([128, 4 * 128], FP32, name=f"pTc_{idx}", tag="b1")
        pTu = pp.tile([128, 4 * 128], FP32, name=f"pTu_{idx}", tag="b2")
        for j in range(4):
            nc.tensor.transpose(
                pTc[:, 128 * j:128 * (j + 1)], cur[:, 128 * j:128 * (j + 1)],
                ident128[:])
            nc.tensor.transpose(
                pTu[:, 128 * j:128 * (j + 1)], U[:, 128 * j:128 * (j + 1)],
                ident128[:])
        curT = sb.tile([128, 4 * 128], FP32, name=f"curT_{idx}", tag="curT")
        UT = sb.tile([128, 4 * 128], FP32, name=f"UT_{idx}", tag="UT")
        nc.scalar.activation(r(curT[:]), pTc[:], AF.Copy)
        nc.vector.tensor_copy(r(UT[:]), pTu[:])
        # P' = sum_t U x cur  (per-group partials on diag blocks)
        psG = pp.tile([128, 128], FP32, name=f"psG_{idx}", tag="b4",
                      padded_shape=[128, L])
        for j in range(4):
            mmr(psG[:], UT[:, 128 * j:128 * (j + 1)],
                curT[:, 128 * j:128 * (j + 1)],
                start=(j == 0), stop=(j == 3))
        Pm = sb.tile([128, 128], FP32, name=f"Pm_{idx}", tag="Pm")
        nc.vector.tensor_tensor(r(Pm[:]), psG[:], mask_diag[:], OP.mult)
        # G_spread = Phi^T (P_m Phi);  P_m = Pm^T
        psM = pp.tile([128, 128], FP32, name=f"psM_{idx}", tag="b5",
                      padded_shape=[128, L])
        mmr(psM[:], Pm[:], phi[:])
        Ms = sb.tile([128, 128], FP32, name=f"Ms_{idx}", tag="Ms")
        nc.scalar.activation(r(Ms[:]), psM[:], AF.Copy)
        psG2 = pp.tile([128, 128], FP32, name=f"psG2_{idx}", tag="b6",
                       padded_shape=[128, L])
        mmr(psG2[:], phi[:], Ms[:])
        Gf = sb.tile([128, 128], FP32, name=f"Gf_{idx}", tag="Gf")
        nc.vector.tensor_tensor(r(Gf[:]), psG2[:], mask_diag[:], OP.mult)
        # xG
        psXG = pp.tile([128, L], FP32, name=f"psXG_{idx}", tag="b6")
        mmr(psXG[:], Gf[:], cur[:])
        # out = (xG - s*U)*a + cur
        sbc_s = sb.tile([128, L], FP32, name=f"sbcs_{idx}", tag="sbcs")
        nc.scalar.activation(sbc_s[:], Sbc[:], AF.Copy)
        w1 = sb.tile([128, L], FP32, name=f"w1_{idx}", tag="w1")
        nc.gpsimd.tensor_tensor(w1[:], sbc_s[:], U[:], OP.mult)
        w2 = sb.tile([128, L], FP32, name=f"w2_{idx}", tag="w2")
        nc.vector.tensor_tensor(w2[:], psXG[:], w1[:], OP.subtract)
        nxt = sb.tile([128, L], FP32, name=f"nxt_{idx}", tag="nxt", bufs=2)
        nc.vector.scalar_tensor_tensor(
            r(nxt[:]), w2[:], at[:], cur[:], OP.mult, OP.add)
        return nxt

    # ================= network ===============================================
    st1 = bn_send(fc(WBD["fc1_w"], X, "1", plain=True), BIAS["fc1_b"][:], "bn1")
    # bn1 AllGather window: queue SWDGE loads + finish fc2/fc3/u1 params
    for w in ("fc2_w", "fc3_w", "u1_w"):
        load_wc(w)
    for b in ("fc2_b", "fc3_b", "u1_b"):
        load_bcv(b)
    load_bn_vecs("bn2")
    build_deferred_consts()
    finish_weight("fc2_w")
    finish_bias("fc2_b")
    finish_weight("fc3_w")
    finish_bias("fc3_b")
    finish_weight("u1_w")
    finish_bias("u1_b")
    build_bnb_bc("bn2")
    h1n = bn_recv(st1, "bn1", rr96, "bn1")
    st2 = bn_send(fc(WBD["fc2_w"], h1n, "2"), BIAS["fc2_b"][:], "bn2")
    # bn2 AllGather window: SWDGE loads first, then u-relmod param finishes
    # (their DMAs land mid-window, before bn2's rank matmuls need the PE)
    emit_relmod_scales()
    for w in ("u2_w", "u3_w", "u4_w"):
        load_wc(w)
    for b in ("u2_b", "u3_b", "u4_b"):
        load_bcv(b)
    for i in (2, 3, 4):
        finish_weight(f"u{i}_w")
        finish_bias(f"u{i}_b")
    h2n = bn_recv(st2, "bn2", rr192, "bn2")
    ps3 = fc(WBD["fc3_w"], h2n, "3")
    enc_r = sb.tile([128, L], FP32, name="enc_r")
    if SAFE_SIGMOID:
        nc.scalar.activation(enc_r[:], ps3[:], AF.Sigmoid, bias=BIAS["fc3_b"][:])
    else:
        # sigmoid(z) = 1/(1+exp(-z)) - keeps ACT on one table set
        b3neg = cp.tile([128, 1], FP32, name="b3neg")
        nc.vector.tensor_scalar_mul(b3neg[:], BIAS["fc3_b"][:], -1.0)
        ex = sb.tile([128, L], FP32, name="ex")
        nc.scalar.activation(ex[:], ps3[:], AF.Exp, scale=-1.0, bias=b3neg[:])
        exp1 = sb.tile([128, L], FP32, name="exp1")
        nc.vector.tensor_scalar_add(exp1[:], ex[:], 1.0)
        rec_scr = sb.tile([128, L], FP32, name="rec_scr")
        nc.vector.reciprocal_approx_accurate(enc_r[:], exp1[:], rec_scr[:])
    # zero the c>=12 garbage rows (sigmoid(0)=0.5) so downstream sums are clean
    enc = sb.tile([128, L], FP32, name="enc")
    nc.vector.tensor_scalar_mul(r(enc[:]), enc_r[:], colmask12[:])

    cur = enc
    for i in range(4):
        cur = relmod(cur, WBD[f"u{i + 1}_w"], BIAS[f"u{i + 1}_b"][:], a_r[i], i)
        if i == 0:
            for w in ("fc4_w", "fc5_w", "fc67_w"):
                load_wc(w)
            for b in ("fc4_b", "fc5_b", "fc67_b"):
                load_bcv(b)
            load_bn_vecs("bn4")
        elif i == 2:
            finish_weight("fc4_w")
            finish_bias("fc4_b")
            build_bnb_bc("bn4")

    st4 = bn_send(fc(WBD["fc4_w"], cur, "4"), BIAS["fc4_b"][:], "bn4")
    finish_weight("fc5_w")
    finish_bias("fc5_b")
    finish_weight("fc67_w")
    finish_bias("fc67_b")
    h4n = bn_recv(st4, "bn4", rr96, "bn4")
    ps5 = fc(WBD["fc5_w"], h4n, "5")
    h5 = sb.tile([128, L], FP32, name="h5")
    nc.scalar.activation(r(h5[:]), ps5[:], AF.Relu, bias=BIAS["fc5_b"][:])
    ps6 = fc(WBD["fc67_w"], h5, "6")
    outs = sb.tile([128, L], FP32, name="outs")
    nc.scalar.add(outs[:], ps6[:], BIAS["fc67_b"][:])

    for g in range(NG):
        b, n0 = g // 4, (g % 4) * L
        eng = nc.sync if g % 2 == 0 else nc.scalar
        eng.dma_start(out_d[b, n0:n0 + L, :].rearrange("n c -> c n"),
                      outs[GS * g:GS * g + F, :])


_PROGRAM = None


def _get_program():
    global _PROGRAM
    if _PROGRAM is None:
        _PROGRAM = _build()
    return _PROGRAM


def run(inputs, trace=False, **kw):
    inputs = {k: np.ascontiguousarray(np.asarray(v, np.float32))
              for k, v in inputs.items()}
    nc = _get_program()
    in_maps = []
    for i in range(NCORES):
        m = {name: inputs[name] for name, _ in PARAM_SPECS}
        m["x"] = np.ascontiguousarray(inputs["x"][BPC * i:BPC * (i + 1)])
        in_maps.append(m)
    last_exc = None
    for attempt in range(3):
        try:
            res = run_bass_kernel_spmd(
                nc, in_maps, core_ids=list(range(NCORES)), trace=trace, **kw)
            break
        except Exception as e:  # transient NRT_EXEC_UNIT_UNRECOVERABLE flakes
            last_exc = e
            import time
            time.sleep(5)
    else:
        raise last_exc
    out = np.concatenate([res.results[i]["out"] for i in range(NCORES)], axis=0)
    return out, res


def kernel(**inputs) -> np.ndarray:
    out, _ = run(inputs)
    return out



# revision 72
# speedup vs baseline: 1.3342x; 1.3342x over previous
"""Trainium2 Bass kernel for nn_Generator_34127810134219 (gnn_message_passing).

Strategy (v3 - replicated head, consolidated params)
----------------------------------------------------
The reference relmod builds a [B,N,N] score matrix S = c*x@x^T (diag masked)
and computes wr*(S@U)/N + x.  Algebraically:

    S@U = c*( x @ (x^T U) - ||x_i||^2 * U_i )

which collapses O(B*N^2*D) work into O(B*N*D^2).

The baseline sharded batch 2-per-core and exchanged BatchNorm statistics via
three AllGathers - each round trip costs ~15-20us of latency.  This version
REPLICATES the head (fc1 -> bn1 -> fc2 -> bn2 stats) on every core over the
FULL batch: every core receives the full x (393KB - tiny), computes global
bn1/bn2 statistics locally, and only then drops to its own 2-batch slice for
the relmods.  Only bn4 (whose input depends on the distributed relmod
outputs) keeps a collective.

Full-width layout: [128, 4096] = 8 column-blocks of 512; column-block j holds
batch pair j in the local layout (partition group g = 4*b_parity + n_window,
slot c, free u; token n = 512*n_window + u).  Per-core inputs are
batch-pair-ROLLED so every core's own pair is column-block 0 - chunk selection
is a compile-time slice and BN sums are permutation-invariant.  BN stats come
from onesfold pack-matmuls accumulated across the 8 column blocks in PSUM.

All parameters are HOST-PACKED into two tensors (pka [12,171], pkbn [24,512])
and the input into x [24,4096], so the kernel issues only ~18 contiguous
DMAs total: SWDGE descriptor generation (~1us each on the Pool engine) and
non-contiguous element-gather DMAs are eliminated entirely.  Weight prep
runs on the otherwise-idle Pool engine during pass A.
"""

import numpy as np

import concourse.bass as bass
import concourse.bacc as bacc
import concourse.tile as tile
import concourse.mybir as mybir
from concourse.bass_utils import run_bass_kernel_spmd
from concourse.masks import make_identity

FP32 = mybir.dt.float32
F32R = mybir.dt.float32r
BF16 = mybir.dt.bfloat16
AF = mybir.ActivationFunctionType
OP = mybir.AluOpType

B, N, F = 16, 2048, 3
D2, D4 = 6, 12
NCORES = 8
BPC = B // NCORES          # batches per core (own pair)
NG = 8                     # partition groups
L = 512                    # free-dim length per column block
W = L * NG                 # full width (4096) = 8 column blocks
GS = 16                    # partition stride per group
EPS = 1e-5

# packed-parameter layout
W_SLOTS = ["fc1_w", "fc2_w", "fc3_w", "u1_w", "u2_w", "u3_w", "u4_w",
           "fc4_w", "fc5_w", "fc67_w"]
B_SLOTS = ["fc1_b", "fc2_b", "fc3_b", "u1_b", "u2_b", "u3_b", "u4_b",
           "fc4_b", "fc5_b", "fc67_b"]
BN_SLOTS = ["bn1_g", "bn1_b", "bn2_g", "bn2_b", "bn4_g", "bn4_b"]
PKA_W = 16 * len(W_SLOTS)              # 160
PKA_B = PKA_W + len(B_SLOTS)           # 170
PKA_COLS = PKA_B + 12                  # 182 (12 relmod scalars on row 0)

PARAM_NAMES = [
    "fc1_w", "fc1_b", "bn1_g", "bn1_b", "fc2_w", "fc2_b", "bn2_g", "bn2_b",
    "fc3_w", "fc3_b",
    "u1_w", "u1_b", "ps1", "ph1", "wr1", "u2_w", "u2_b", "ps2", "ph2", "wr2",
    "u3_w", "u3_b", "ps3", "ph3", "wr3", "u4_w", "u4_b", "ps4", "ph4", "wr4",
    "fc4_w", "fc4_b", "bn4_g", "bn4_b", "fc5_w", "fc5_b",
    "fc6_w", "fc6_b", "fc7_w", "fc7_b",
]


def _build(single_core=False):
    nc = bacc.Bacc(
        "TRN2",
        target_bir_lowering=False,
        debug=False,
        enable_asserts=False,
        num_devices=1 if single_core else NCORES,
    )

    # host-packed inputs (see _pack_x/_pack_params)
    x_d = nc.dram_tensor("x", [24, W], F32R, kind="ExternalInput")
    pka_d = nc.dram_tensor("pka", [D4, PKA_COLS], FP32, kind="ExternalInput")
    # pkbn: 3 stacked [4,1024] blocks (bnX_b cols 0-511, bnX_g cols 512-1023),
    # each loaded into its own base-0 tile (engine operands must start at
    # partition 0)
    pkbn_d = nc.dram_tensor("pkbn", [12, 2 * L], FP32, kind="ExternalInput")
    # out[3(4b+w)+c, u] = out_full[b, 512w+u, c]  (own pair, host-unpacked)
    out_d = nc.dram_tensor("out", [24, L], FP32, kind="ExternalOutput")

    with tile.TileContext(nc) as tc:
        with (
            tc.tile_pool(name="consts", bufs=1) as cp,
            tc.tile_pool(name="sb", bufs=1) as sb,
            tc.tile_pool(name="pp", bufs=1, space="PSUM") as pp,
            tc.tile_pool(name="dram", bufs=1, space="DRAM") as dr,
        ):
            _emit(nc, tc, cp, sb, pp, dr, x_d, pka_d, pkbn_d, out_d,
                  single_core=single_core)

    nc.compile()
    return nc


def _emit(nc, tc, cp, sb, pp, dr, x_d, pka_d, pkbn_d, out_d, single_core=False):
    def mmr(out, lhsT, rhs, **kw):
        """float32r matmul: full-rate PE for fp32 bits (reduced mult precision)."""
        nc.tensor.matmul(out, lhsT.bitcast(F32R), rhs.bitcast(F32R), **kw)

    def r(ap):
        """f32r view for producer outputs feeding f32r matmuls (rounds)."""
        return ap.bitcast(F32R)

    # ---- the two packed parameter DMAs + input chunk DMAs, split so the
    # last X2 chunks don't queue behind the first four (pass A consumes them
    # at ~0.9us cadence)
    pka = cp.tile([D4, PKA_COLS], FP32, name="pka")
    nc.scalar.dma_start(pka[:], pka_d[:, :])
    X2 = sb.tile([24, W], F32R, name="X2")
    for j in range(6):
        nc.sync.dma_start(X2[:, L * j:L * (j + 1)], x_d[:, L * j:L * (j + 1)])
    for j in (6, 7):
        nc.scalar.dma_start(X2[:, L * j:L * (j + 1)], x_d[:, L * j:L * (j + 1)])
    pkb = {}
    for i, k in enumerate(("bn1", "bn2", "bn4")):
        pkb[k] = cp.tile([4, 2 * L], FP32, name=f"pkb_{k}")
        nc.sync.dma_start(pkb[k][:], pkbn_d[4 * i:4 * i + 4, :])

    # parameter views into the packed tiles
    _WC = {nm: pka[:, 16 * k:16 * (k + 1)] for k, nm in enumerate(W_SLOTS)}
    _BCV = {nm: pka[:, PKA_W + k:PKA_W + k + 1] for k, nm in enumerate(B_SLOTS)}
    bnb = {k: pkb[k][:, 0:L] for k in pkb}
    bng = {k: pkb[k][:, L:2 * L] for k in pkb}

    eps_t = cp.tile([128, 1], FP32, name="eps_t")
    nc.gpsimd.memset(eps_t[:], EPS)
    # first ACT instruction resolves the abs_rsqrt table set up front
    actwarm = sb.tile([1, 1], FP32, name="actwarm")
    nc.scalar.activation(actwarm[:], eps_t[0:1, :], AF.Abs_reciprocal_sqrt)

    # ================= affine-built base selectors (Pool engine) =============
    def affine_sel(t, pattern, cm):
        """t := 1.0 where cm*p + pattern.idx == 0 else 0."""
        nc.gpsimd.memset(t, 0.0)
        nc.gpsimd.affine_select(
            out=t, in_=t, compare_op=OP.not_equal, fill=1.0,
            base=0, pattern=pattern, channel_multiplier=cm)

    # selectors needed by the fc1 weight build come first
    bc8 = cp.tile([NG, 128], FP32, name="bc8")
    affine_sel(bc8[:].rearrange("p (g c) -> p g c", c=GS), [[1, NG], [0, GS]], -1)
    bc83 = cp.tile([NG, 24], FP32, name="bc83")
    affine_sel(bc83[:].rearrange("p (g c) -> p g c", c=3), [[1, NG], [0, 3]], -1)
    rsel3 = cp.tile([3, 24], FP32, name="rsel3")
    affine_sel(rsel3[:].rearrange("p (g c) -> p g c", c=3), [[0, NG], [1, 3]], -1)
    rsel16 = cp.tile([GS, 128], FP32, name="rsel16")
    affine_sel(rsel16[:].rearrange("p (g c) -> p g c", c=GS), [[0, NG], [1, GS]], -1)
    bc4 = cp.tile([4, 128], FP32, name="bc4")
    affine_sel(bc4[:].rearrange("p (h j c) -> p h j c", j=4, c=GS),
               [[0, 2], [1, 4], [0, GS]], -1)
    bcB = cp.tile([2, 128], FP32, name="bcB")
    affine_sel(bcB[:].rearrange("p (b j c) -> p b j c", j=4, c=GS),
               [[1, 2], [0, 4], [0, GS]], -1)
    rsel12 = cp.tile([D4, 128], FP32, name="rsel12")
    affine_sel(rsel12[:].rearrange("p (g c) -> p g c", c=GS), [[0, NG], [1, GS]], -1)
    s8 = cp.tile([8, 64], FP32, name="s8")
    affine_sel(s8[:].rearrange("p (r j) -> p r j", j=8), [[0, 8], [1, 8]], -1)
    # sel16_24[c', (g,co)] = [co==c']  (compact fc67 output columns)
    sel1624 = cp.tile([GS, 24], FP32, name="sel1624")
    affine_sel(sel1624[:].rearrange("p (g c) -> p g c", c=3), [[0, NG], [1, 3]], -1)

    ident128 = cp.tile([128, 128], FP32, name="ident128")
    make_identity(nc, ident128[:])
    ones1 = cp.tile([1, 128], FP32, name="ones1")
    nc.gpsimd.memset(ones1[:], 1.0)
    ones12 = cp.tile([D4, 1], FP32, name="ones12")
    nc.gpsimd.memset(ones12[:], 1.0)
    c96 = cp.tile([4, 1], FP32, name="c96")
    nc.gpsimd.memset(c96[:], 1.0 / 96.0)
    c192 = cp.tile([4, 1], FP32, name="c192")
    nc.gpsimd.memset(c192[:], 1.0 / 192.0)
    cneg1 = cp.tile([4, 1], FP32, name="cneg1")
    nc.gpsimd.memset(cneg1[:], -1.0)

    # ================= PE-derived constant tiles =============================
    # mask24[(g,ci),(g',c')] = [g==g']  (fc1 weight mask - first, it gates fc1)
    m24_ps = pp.tile([24, 128], FP32, name="m24_ps", tag="b1", padded_shape=[128, L])
    nc.tensor.matmul(m24_ps[:], bc83[:], bc8[:])
    mask24 = cp.tile([24, 128], FP32, name="mask24")
    nc.vector.tensor_copy(mask24[:], m24_ps[:])
    # mask_diag[(g,c),(g',c')] = [g==g']
    mask_ps = pp.tile([128, 128], FP32, name="mask_ps", tag="b0", padded_shape=[128, L])
    nc.tensor.matmul(mask_ps[:], bc8[:], bc8[:])
    mask_diag = cp.tile([128, 128], FP32, name="mask_diag")
    nc.vector.tensor_copy(mask_diag[:], mask_ps[:])
    # mask128_24[(g,ci),(g',co)] = [g==g']  (fc67 compact-output mask)
    mc_ps = pp.tile([128, 24], FP32, name="mc_ps", tag="b2", padded_shape=[128, L])
    nc.tensor.matmul(mc_ps[:], bc8[:], bc83[:])
    mask12824 = cp.tile([128, 24], FP32, name="mask12824")
    nc.vector.tensor_copy(mask12824[:], mc_ps[:])
    # onesfold [128,4] = bc4^T  (pack matmul lhsT: (g,c) -> n-window)
    of_ps = pp.tile([128, 4], FP32, name="of_ps", tag="b3", padded_shape=[128, L])
    nc.tensor.transpose(of_ps[:], bc4[:], ident128[0:4, 0:4])
    onesfold = cp.tile([128, 4], FP32, name="onesfold")
    nc.vector.tensor_copy(r(onesfold[:]), of_ps[:])
    # phi (batch fold/spread), ones_c16, colmask12
    crep_ps = pp.tile([128, 128], FP32, name="crep_ps", tag="b2",
                      padded_shape=[128, L])
    nc.tensor.matmul(crep_ps[:], rsel16[:], rsel16[:])
    crep = sb.tile([128, 128], FP32, name="crep")
    nc.vector.tensor_copy(crep[:], crep_ps[:])
    bmask_ps = pp.tile([128, 128], FP32, name="bmask_ps", tag="b4",
                       padded_shape=[128, L])
    nc.tensor.matmul(bmask_ps[:], bcB[:], bcB[:])
    phi = cp.tile([128, 128], BF16, name="phi")
    nc.vector.tensor_tensor(phi[:], bmask_ps[:], crep[:], OP.mult)
    oc_ps = pp.tile([128, NG], FP32, name="oc_ps", tag="b5",
                    padded_shape=[128, L])
    nc.tensor.transpose(oc_ps[:], bc8[:], ident128[0:NG, 0:NG])
    ones_c16 = cp.tile([128, NG], BF16, name="ones_c16")
    nc.vector.tensor_copy(ones_c16[:], oc_ps[:])
    cm_ps = pp.tile([128, 1], FP32, name="cm_ps", tag="b6",
                    padded_shape=[128, L])
    nc.tensor.matmul(cm_ps[:], rsel12[:], ones12[:])
    colmask12 = cp.tile([128, 1], FP32, name="colmask12")
    nc.vector.tensor_copy(colmask12[:], cm_ps[:])
    # f32r-rounded copy of bc4; bf16 copies of bc8/ident for the relmod path
    bc4r = cp.tile([4, 128], FP32, name="bc4r")
    nc.vector.tensor_copy(r(bc4r[:]), bc4[:])
    bc8b = cp.tile([NG, 128], BF16, name="bc8b")
    nc.vector.tensor_copy(bc8b[:], bc8[:])
    ident_b = cp.tile([128, 128], BF16, name="ident_b")
    nc.vector.tensor_copy(ident_b[:], ident128[:])
    # rank-reduction tile [64,8] for the bn4 AllGather (scaled 1/96)
    rr_ps = pp.tile([64, 8], FP32, name="rr_ps", tag="b7", padded_shape=[128, L])
    nc.tensor.transpose(rr_ps[:], s8[:], ident128[0:8, 0:8])
    rr96 = cp.tile([64, 8], FP32, name="rr96")
    nc.vector.tensor_scalar_mul(r(rr96[:]), rr_ps[:], 1.0 / 96.0)

    # ================= weights / biases ======================================
    # finish_* run tp/sp on PE and the copies/masks on the idle Pool engine
    WBD = {}
    BIAS = {}
    _wtag = [0]

    def finish_weight(wname, dt=FP32):
        tag1, tag2 = ("b4", "b7") if _wtag[0] % 2 == 0 else ("b7", "b4")
        _wtag[0] += 1
        tp = pp.tile([GS, 128], FP32, name=f"wt_{wname}", tag=tag1,
                     padded_shape=[128, L])
        nc.tensor.matmul(tp[:], _WC[wname], rsel12[:])
        ts = sb.tile([GS, 128], FP32, name=f"ws_{wname}", tag="wts", bufs=2)
        nc.scalar.activation(ts[:], tp[:], AF.Copy)
        sp = pp.tile([128, 128], FP32, name=f"wsp_{wname}", tag=tag2,
                     padded_shape=[128, L])
        nc.tensor.matmul(sp[:], ts[:], rsel16[:])
        wt = cp.tile([128, 128], dt, name=f"W_{wname}")
        out = r(wt[:]) if dt == FP32 else wt[:]
        nc.vector.tensor_tensor(out, sp[:], mask_diag[:], OP.mult)
        WBD[wname] = wt

    def finish_bias(bname):
        tag = "b4" if _wtag[0] % 2 == 0 else "b7"
        _wtag[0] += 1
        bps = pp.tile([128, 1], FP32, name=f"bps_{bname}", tag=tag,
                      padded_shape=[128, L])
        nc.tensor.matmul(bps[:], rsel12[:], _BCV[bname])
        bt = cp.tile([128, 1], FP32, name=f"bias_{bname}")
        nc.scalar.activation(bt[:], bps[:], AF.Copy)
        BIAS[bname] = bt

    # fc1: 24-row block-diag [24, 128]: W1bd[(g,ci),(g',co)] = [g==g'] W1[co,ci]
    tp1 = pp.tile([GS, 24], FP32, name="wt1", tag="b6", padded_shape=[128, L])
    nc.tensor.matmul(tp1[:], pka[0:3, 0:GS], rsel3[:])
    ts1 = sb.tile([GS, 24], FP32, name="ws1", tag="wts", bufs=2)
    nc.scalar.activation(ts1[:], tp1[:], AF.Copy)
    sp1 = pp.tile([24, 128], FP32, name="wsp1", tag="b7", padded_shape=[128, L])
    nc.tensor.matmul(sp1[:], ts1[:], rsel16[:])
    w1t = cp.tile([24, 128], FP32, name="W_fc1")
    nc.vector.tensor_tensor(r(w1t[:]), sp1[:], mask24[:], OP.mult)
    WBD["fc1_w"] = w1t
    finish_bias("fc1_b")

    # relmod scale values wr*ps*ph/N (tiny DVE chains; broadcast comes later)
    at_s = []
    for i in range(4):
        t = sb.tile([1, 1], FP32, name=f"atsc_{i}", tag=f"sc{i}")
        c0 = PKA_B + 3 * i
        nc.vector.tensor_tensor(t[:], pka[0:1, c0:c0 + 1],
                                pka[0:1, c0 + 1:c0 + 2], OP.mult)
        nc.vector.tensor_tensor(t[:], t[:], pka[0:1, c0 + 2:c0 + 3], OP.mult)
        nc.vector.tensor_scalar_mul(t[:], t[:], 1.0 / N)
        at_s.append(t)

    # ================= replicated head =======================================
    # pass A: fc1 over all 8 column blocks + bn1 stats accumulation.
    # PSUM banks: b0-b3 (fc1 rotation), b5 (sum), b6 (sumsq), b4/b7 (weights).
    hs1 = sb.tile([128, W], FP32, name="hs1")
    psA = {}

    def fc1_mm(j):
        psA[j] = pp.tile([128, L], FP32, name=f"psA_{j}", tag=f"b{j % 4}")
        mmr(psA[j][:], WBD["fc1_w"][:], X2[:, L * j:L * (j + 1)])

    for j in range(4):
        fc1_mm(j)
    st1_s = pp.tile([4, L], FP32, name="st1_s", tag="b5", padded_shape=[128, L])
    st1_q = pp.tile([4, L], FP32, name="st1_q", tag="b6", padded_shape=[128, L])
    for j in range(NG):
        hj = hs1[:, L * j:L * (j + 1)]
        nc.scalar.add(r(hj), psA[j][:], BIAS["fc1_b"][:])
        sq = sb.tile([128, L], FP32, name=f"sq1_{j}", tag=f"sq{j % 2}")
        nc.vector.tensor_tensor(r(sq[:]), hj, hj, OP.mult)
        mmr(st1_s[:], onesfold[:], hj, start=(j == 0), stop=(j == NG - 1))
        mmr(st1_q[:], onesfold[:], sq[:], start=(j == 0), stop=(j == NG - 1))
        if j + 4 < NG:
            fc1_mm(j + 4)

    # bn stats chain: a = g/sqrt(var+eps), d = bnb - a*mean (broadcast tiles)
    def stats_chain(st_s, st_q, cinv, cp_t, key, tag, sbuf_d=True):
        mean = sb.tile([4, L], FP32, name=f"mean_{tag}")
        nc.scalar.activation(mean[:], st_s[:], AF.Copy, scale=float(cinv))
        msq = sb.tile([4, L], FP32, name=f"msq_{tag}")
        nc.scalar.activation(msq[:], st_s[:], AF.Square, scale=float(cinv))
        var = sb.tile([4, L], FP32, name=f"var_{tag}")
        nc.vector.scalar_tensor_tensor(var[:], st_q[:], cp_t[:], msq[:],
                                       OP.mult, OP.subtract)
        rstd = sb.tile([4, L], FP32, name=f"rstd_{tag}")
        nc.scalar.activation(rstd[:], var[:], AF.Abs_reciprocal_sqrt,
                             bias=eps_t[0:4, :])
        a = sb.tile([4, L], FP32, name=f"a_{tag}")
        nc.vector.tensor_tensor(r(a[:]), rstd[:], bng[key], OP.mult)
        amn = sb.tile([4, L], FP32, name=f"amn_{tag}")
        nc.vector.scalar_tensor_tensor(r(amn[:]), a[:], cneg1[:], mean[:],
                                       OP.mult, OP.mult)
        # broadcasts: Abc = a per (window,u); Dbc = bnb + (-a*mean)
        Abc = pp.tile([128, L], FP32, name=f"Abc_{tag}", tag="b3")
        mmr(Abc[:], bc4r[:], a[:])
        Dbc = pp.tile([128, L], FP32, name=f"Dbc_{tag}", tag="b4")
        nc.tensor.matmul(Dbc[:], bc4[:], bnb[key], start=True, stop=False)
        mmr(Dbc[:], bc4r[:], amn[:], start=False, stop=True)
        if not sbuf_d:
            return Abc, Dbc
        # Pool (gpsimd) has no PSUM access - give it an SBUF copy of Dbc
        Dbc_s = sb.tile([128, L], FP32, name=f"Dbcs_{tag}")
        nc.vector.tensor_copy(Dbc_s[:], Dbc[:])
        return Abc, Dbc_s

    Abc1, Dbc1 = stats_chain(st1_s, st1_q, 1.0 / 96.0, c96, "bn1", "bn1")

    # finish the head/relmod weights inside the bn1-chain and pass-B windows
    # (their ACT/DVE copies ride in the dependency-wait gaps)
    for wn, bn_, wdt in (("fc2_w", "fc2_b", FP32), ("fc3_w", "fc3_b", FP32),
                         ("u1_w", "u1_b", BF16), ("u2_w", "u2_b", BF16),
                         ("u3_w", "u3_b", BF16)):
        finish_weight(wn, wdt)
        finish_bias(bn_)

    # pass B: bn1 apply + relu + fc2 + bn2 stats accumulation.
    # PSUM banks: b0/b1 (fc2 rotation), b5 (sum), b6 (sumsq), b3 (Abc1),
    # b4 (bnb_bc for bn4, after Dbc1 is copied out).
    hs2 = sb.tile([128, W], FP32, name="hs2")
    st2_s = pp.tile([4, L], FP32, name="st2_s", tag="b5", padded_shape=[128, L])
    st2_q = pp.tile([4, L], FP32, name="st2_q", tag="b6", padded_shape=[128, L])
    for j in range(NG):
        t2 = sb.tile([128, L], FP32, name=f"t2_{j}", tag=f"t2{j % 2}")
        nc.vector.tensor_tensor(t2[:], hs1[:, L * j:L * (j + 1)],
                                Abc1[:], OP.mult)
        t3 = sb.tile([128, L], FP32, name=f"t3_{j}", tag=f"t3{j % 2}")
        nc.gpsimd.tensor_tensor(t3[:], t2[:], Dbc1[:], OP.add)
        rl = sb.tile([128, L], FP32, name=f"rl_{j}", tag=f"rl{j % 2}")
        nc.scalar.activation(r(rl[:]), t3[:], AF.Relu)
        psB = pp.tile([128, L], FP32, name=f"psB_{j}", tag=f"b{j % 2}")
        mmr(psB[:], WBD["fc2_w"][:], rl[:])
        h2j = hs2[:, L * j:L * (j + 1)]
        nc.scalar.add(r(h2j), psB[:], BIAS["fc2_b"][:])
        sq = sb.tile([128, L], FP32, name=f"sq2_{j}", tag=f"sq{j % 2}")
        nc.vector.tensor_tensor(r(sq[:]), h2j, h2j, OP.mult)
        mmr(st2_s[:], onesfold[:], h2j, start=(j == 0), stop=(j == NG - 1))
        mmr(st2_q[:], onesfold[:], sq[:], start=(j == 0), stop=(j == NG - 1))
        if j == 1:
            finish_weight("u4_w", BF16)
            finish_bias("u4_b")
        elif j == 3:
            finish_weight("fc4_w", BF16)
            finish_bias("fc4_b")
        elif j == 5:
            finish_weight("fc5_w", FP32)
            finish_bias("fc5_b")
        elif j == 7:
            # fc67 compact: lhsT [128,24] keeping only the 3 real output
            # cols per group, so the output tile is [24,512] -> one DMA
            tp67 = pp.tile([GS, 128], FP32, name="wt_fc67", tag="b4",
                           padded_shape=[128, L])
            nc.tensor.matmul(tp67[:], _WC["fc67_w"], rsel12[:])
            ts67 = sb.tile([GS, 128], FP32, name="ws_fc67", tag="wts", bufs=2)
            nc.scalar.activation(ts67[:], tp67[:], AF.Copy)
            sp67 = pp.tile([128, 24], FP32, name="wsp_fc67", tag="b7",
                           padded_shape=[128, L])
            nc.tensor.matmul(sp67[:], ts67[:], sel1624[:])
            w67c = cp.tile([128, 24], FP32, name="W_fc67c")
            nc.vector.tensor_tensor(r(w67c[:]), sp67[:], mask12824[:], OP.mult)
            WBD["fc67_w"] = w67c
            b67_ps = pp.tile([24, 1], FP32, name="b67_ps", tag="b4",
                             padded_shape=[128, L])
            nc.tensor.matmul(b67_ps[:], rsel3[:], _BCV["fc67_b"][0:3, 0:1])
            b67c = cp.tile([24, 1], FP32, name="bias24")
            nc.scalar.activation(b67c[:], b67_ps[:], AF.Copy)
            BIAS["fc67_b"] = b67c

    Abc2, Dbc2 = stats_chain(st2_s, st2_q, 1.0 / 192.0, c192, "bn2", "bn2",
                             sbuf_d=False)

    # relmod scale broadcasts [128,1] (b6 free between st2_q and the relmods)
    a_r = []
    for i in range(4):
        pb = pp.tile([128, 1], FP32, name=f"psc_{i}", tag="b6",
                     padded_shape=[128, L])
        nc.tensor.matmul(pb[:], ones1[:], at_s[i][:])
        at = cp.tile([128, 1], FP32, name=f"a_r{i}")
        nc.vector.tensor_copy(at[:], pb[:])
        a_r.append(at)

    # local phase: bn2 apply on own chunk (column block 0) + fc3 + sigmoid.
    # All three pointwise ops on DVE: no cross-engine hops, and Pool ALU ops
    # run at 0.42 efficiency (~1.1us per [128,512] op).
    t2o = sb.tile([128, L], FP32, name="t2o")
    nc.vector.tensor_tensor(t2o[:], hs2[:, 0:L], Abc2[:], OP.mult)
    t3o = sb.tile([128, L], FP32, name="t3o")
    nc.vector.tensor_tensor(t3o[:], t2o[:], Dbc2[:], OP.add)
    rlo = sb.tile([128, L], FP32, name="rlo")
    nc.vector.tensor_relu(r(rlo[:]), t3o[:])
    ps3 = pp.tile([128, L], FP32, name="psfc_3", tag="b0")
    mmr(ps3[:], WBD["fc3_w"][:], rlo[:])
    enc_r = sb.tile([128, L], FP32, name="enc_r")
    nc.scalar.activation(enc_r[:], ps3[:], AF.Sigmoid, bias=BIAS["fc3_b"][:])
    # zero the c>=12 garbage rows (sigmoid(0)=0.5) so downstream sums are
    # clean; bf16 - the whole relmod pipeline runs in bf16
    enc = sb.tile([128, L], BF16, name="enc")
    nc.vector.tensor_scalar_mul(enc[:], enc_r[:], colmask12[:])

    # bnb broadcast for the bn4 recv path (Pool is idle here; b4 is free
    # once the local phase has read Dbc2)
    bnbps4 = pp.tile([128, L], FP32, name="bnbps_bn4", tag="b4")
    nc.tensor.matmul(bnbps4[:], bc4[:], bnb["bn4"])
    bnbbc4 = cp.tile([128, L], FP32, name="bnbbc_bn4")
    nc.vector.tensor_copy(bnbbc4[:], bnbps4[:])

    # ================= relmods ===============================================
    def fc(w, src, name):
        ps = pp.tile([128, L], FP32, name=f"psfc_{name}", tag="b0")
        mmr(ps[:], w[:], src[:])
        return ps

    def relmod(cur, wu, bu, at, idx):
        """cur/U/Gram pipeline all in bf16: transposes and 128-col matmuls run
        at 1 cycle/row (vs 2/4 for f32), pure-bf16 DVE ops at 2x."""
        psU = pp.tile([128, L], FP32, name=f"psU_{idx}", tag="b0")
        nc.tensor.matmul(psU[:], wu[:], cur[:])
        U = sb.tile([128, L], BF16, name=f"U_{idx}", tag="U")
        nc.scalar.activation(U[:], psU[:], AF.Relu, bias=bu)
        # s = sum_c cur^2 per token, broadcast to [128,L]
        sq = sb.tile([128, L], BF16, name=f"rsq_{idx}", tag="rsq")
        nc.scalar.activation(sq[:], cur[:], AF.Square)
        psS = pp.tile([NG, L], FP32, name=f"psS_{idx}", tag="b5", padded_shape=[128, L])
        nc.tensor.matmul(psS[:], ones_c16[:], sq[:])
        sS = sb.tile([NG, L], BF16, name=f"sS_{idx}", tag="sS")
        nc.vector.tensor_copy(sS[:], psS[:])
        Sbc = pp.tile([128, L], FP32, name=f"Sbc_{idx}", tag="b3")
        nc.tensor.matmul(Sbc[:], bc8b[:], sS[:])
        # transposes of cur and U (4x 128-chunks each)
        pTc = pp.tile([128, 4 * 128], BF16, name=f"pTc_{idx}", tag="b1",
                      padded_shape=[128, 2 * L])
        pTu = pp.tile([128, 4 * 128], BF16, name=f"pTu_{idx}", tag="b2",
                      padded_shape=[128, 2 * L])
        for j in range(4):
            nc.tensor.transpose(
                pTc[:, 128 * j:128 * (j + 1)], cur[:, 128 * j:128 * (j + 1)],
                ident_b[:])
            nc.tensor.transpose(
                pTu[:, 128 * j:128 * (j + 1)], U[:, 128 * j:128 * (j + 1)],
                ident_b[:])
        curT = sb.tile([128, 4 * 128], BF16, name=f"curT_{idx}", tag="curT")
        UT = sb.tile([128, 4 * 128], BF16, name=f"UT_{idx}", tag="UT")
        nc.scalar.activation(curT[:], pTc[:], AF.Copy)
        nc.vector.tensor_copy(UT[:], pTu[:])
        # P' = sum_t U x cur  (per-group partials on diag blocks)
        psG = pp.tile([128, 128], FP32, name=f"psG_{idx}", tag="b4",
                      padded_shape=[128, L])
        for j in range(4):
            nc.tensor.matmul(psG[:], UT[:, 128 * j:128 * (j + 1)],
                             curT[:, 128 * j:128 * (j + 1)],
                             start=(j == 0), stop=(j == 3))
        Pm = sb.tile([128, 128], BF16, name=f"Pm_{idx}", tag="Pm")
        nc.vector.tensor_tensor(Pm[:], psG[:], mask_diag[:], OP.mult)
        # G_spread = Phi^T (P_m Phi);  P_m = Pm^T
        psM = pp.tile([128, 128], FP32, name=f"psM_{idx}", tag="b5",
                      padded_shape=[128, L])
        nc.tensor.matmul(psM[:], Pm[:], phi[:])
        Ms = sb.tile([128, 128], BF16, name=f"Ms_{idx}", tag="Ms")
        nc.scalar.activation(Ms[:], psM[:], AF.Copy)
        psG2 = pp.tile([128, 128], FP32, name=f"psG2_{idx}", tag="b6",
                       padded_shape=[128, L])
        nc.tensor.matmul(psG2[:], phi[:], Ms[:])
        Gf = sb.tile([128, 128], BF16, name=f"Gf_{idx}", tag="Gf")
        nc.vector.tensor_tensor(Gf[:], psG2[:], mask_diag[:], OP.mult)
        # xG
        psXG = pp.tile([128, L], FP32, name=f"psXG_{idx}", tag="b6")
        nc.tensor.matmul(psXG[:], Gf[:], cur[:])
        # out = (xG - s*U)*a + cur
        sbc_s = sb.tile([128, L], BF16, name=f"sbcs_{idx}", tag="sbcs")
        nc.scalar.activation(sbc_s[:], Sbc[:], AF.Copy)
        w1 = sb.tile([128, L], BF16, name=f"w1_{idx}", tag="w1")
        nc.vector.tensor_tensor(w1[:], sbc_s[:], U[:], OP.mult)
        if idx == 0:
            # prefetch the abs_rsqrt table set (evicted by enc's sigmoid) in
            # the relmod-0 ACT slack so bn4's rstd doesn't pay the 1.3us
            # load; the w1 input pins this after the sigmoid.
            aw2 = sb.tile([1, 1], FP32, name="actwarm2")
            nc.scalar.activation(aw2[:], w1[0:1, 0:1], AF.Abs_reciprocal_sqrt)
        w2 = sb.tile([128, L], BF16, name=f"w2_{idx}", tag="w2")
        nc.vector.tensor_tensor(w2[:], psXG[:], w1[:], OP.subtract)
        nxt = sb.tile([128, L], BF16, name=f"nxt_{idx}", tag="nxt", bufs=2)
        nc.vector.scalar_tensor_tensor(
            nxt[:], w2[:], at[:], cur[:], OP.mult, OP.add)
        return nxt

    cur = enc
    for i in range(4):
        cur = relmod(cur, WBD[f"u{i + 1}_w"], BIAS[f"u{i + 1}_b"][:],
                     a_r[i], i)

    # ================= tail: fc4 + bn4 (the one collective) ==================
    h4_ps = pp.tile([128, L], FP32, name="psfc_4", tag="b0")
    nc.tensor.matmul(h4_ps[:], WBD["fc4_w"][:], cur[:])
    hs4 = sb.tile([128, L], FP32, name="hs_bn4")
    nc.scalar.add(r(hs4[:]), h4_ps[:], BIAS["fc4_b"][:])
    sq4 = sb.tile([128, L], FP32, name="sq_bn4")
    nc.scalar.activation(r(sq4[:]), h4_ps[:], AF.Square, bias=BIAS["fc4_b"][:])
    pk_s = pp.tile([4, L], FP32, name="pks_bn4", tag="b1", padded_shape=[128, L])
    pk_q = pp.tile([4, L], FP32, name="pkq_bn4", tag="b2", padded_shape=[128, L])
    mmr(pk_s[:], onesfold[:], hs4[:])
    mmr(pk_q[:], onesfold[:], sq4[:])
    sk_s = sb.tile([4, L], F32R, name="sks_bn4")
    sk_q = sb.tile([4, L], F32R, name="skq_bn4")
    nc.scalar.activation(sk_s[:], pk_s[:], AF.Copy)
    nc.vector.tensor_copy(sk_q[:], pk_q[:])
    cc_in = dr.tile([8, L], F32R, name="ccin_bn4")
    cc_out = dr.tile([64, L], F32R, name="ccout_bn4")
    nc.sync.dma_start(cc_in[0:4, :], sk_s[:])
    nc.scalar.dma_start(cc_in[4:8, :], sk_q[:])
    if single_core:
        # timing-only stand-in for the AllGather (TimelineSim path);
        # 4 serialized DMAs model the ~5us 8-core AllGather latency
        for rr in range(4):
            nc.sync.dma_start(cc_out[8 * rr:8 * rr + 8, :], cc_in[:])
    else:
        nc.gpsimd.collective_compute(
            "AllGather",
            OP.bypass,
            replica_groups=[list(range(NCORES))],
            ins=[cc_in.opt()],
            outs=[cc_out.opt()],
        )
    gath = sb.tile([64, L], F32R, name="gath_bn4")
    nc.sync.dma_start(gath[:], cc_out[:])
    m_ps = pp.tile([4, L], FP32, name="mps_bn4", tag="b1", padded_shape=[128, L])
    q_ps = pp.tile([4, L], FP32, name="qps_bn4", tag="b2", padded_shape=[128, L])
    mmr(m_ps[:], rr96[:, 0:4], gath[:])
    mmr(q_ps[:], rr96[:, 4:8], gath[:])
    mean4 = sb.tile([4, L], FP32, name="mean_bn4")
    nc.scalar.activation(r(mean4[:]), m_ps[:], AF.Copy)
    Mbc4 = pp.tile([128, L], FP32, name="Mbc_bn4", tag="b4")
    mmr(Mbc4[:], bc4r[:], mean4[:])
    t1_4 = sb.tile([128, L], FP32, name="t1_bn4")
    nc.vector.tensor_tensor(t1_4[:], hs4[:], Mbc4[:], OP.subtract)
    msq4 = sb.tile([4, L], FP32, name="msq_bn4")
    nc.scalar.activation(msq4[:], m_ps[:], AF.Square)
    var4 = sb.tile([4, L], FP32, name="var_bn4")
    nc.vector.tensor_tensor(var4[:], q_ps[:], msq4[:], OP.subtract)
    rstd4 = sb.tile([4, L], FP32, name="rstd_bn4")
    nc.scalar.activation(rstd4[:], var4[:], AF.Abs_reciprocal_sqrt,
                         bias=eps_t[0:4, :])
    a4 = sb.tile([4, L], FP32, name="a_bn4")
    nc.vector.tensor_tensor(r(a4[:]), rstd4[:], bng["bn4"], OP.mult)
    Abc4 = pp.tile([128, L], FP32, name="Abc_bn4", tag="b3")
    mmr(Abc4[:], bc4r[:], a4[:])
    t2_4 = sb.tile([128, L], FP32, name="t2_bn4")
    nc.vector.tensor_tensor(t2_4[:], t1_4[:], Abc4[:], OP.mult)
    t3_4 = sb.tile([128, L], FP32, name="t3_bn4")
    nc.vector.tensor_tensor(t3_4[:], t2_4[:], bnbbc4[:], OP.add)
    hn4 = sb.tile([128, L], FP32, name="hn_bn4")
    nc.vector.tensor_relu(r(hn4[:]), t3_4[:])

    ps5 = fc(WBD["fc5_w"], hn4, "5")
    h5 = sb.tile([128, L], FP32, name="h5")
    nc.scalar.activation(r(h5[:]), ps5[:], AF.Relu, bias=BIAS["fc5_b"][:])
    ps6 = pp.tile([24, L], FP32, name="psfc_6", tag="b0")
    mmr(ps6[:], WBD["fc67_w"][:], h5[:])
    outs = sb.tile([24, L], FP32, name="outs")
    nc.scalar.add(outs[:], ps6[:], BIAS["fc67_b"][:])
    nc.sync.dma_start(out_d[:, :], outs[:])


_PROGRAM = None


def _get_program():
    global _PROGRAM
    if _PROGRAM is None:
        _PROGRAM = _build()
    return _PROGRAM


def _pack_x(x):
    """[16,2048,3] -> [24,4096]: out[3(4p+w)+ci, 512j+u] = x[2j+p, 512w+u, ci]."""
    v = x.reshape(NCORES, 2, 4, L, F).transpose(1, 2, 4, 0, 3)
    return np.ascontiguousarray(v.reshape(24, W))


def _unpack_out(o):
    """[24,512] -> [2,2048,3]: o[3(4b+w)+c, u] = out[b, 512w+u, c]."""
    return np.ascontiguousarray(
        o.reshape(2, 4, F, L).transpose(0, 1, 3, 2).reshape(BPC, N, F))


def _pack_params(inp):
    """Assemble pka [12,171] (weights/biases/scalars) and pkbn [24,512]."""
    pka = np.zeros((D4, PKA_COLS), np.float32)
    for k, nm in enumerate(W_SLOTS):
        if nm == "fc67_w":
            pka[0:F, 16 * k] = inp["fc6_w"][0]
            pka[0:F, 16 * k + 1:16 * k + 3] = inp["fc7_w"].T
        else:
            wt = inp[nm].T  # [in, out]
            pka[0:wt.shape[0], 16 * k:16 * k + wt.shape[1]] = wt
    for k, nm in enumerate(B_SLOTS):
        if nm == "fc67_b":
            pka[0, PKA_W + k] = inp["fc6_b"][0]
            pka[1:3, PKA_W + k] = inp["fc7_b"]
        else:
            bv = inp[nm]
            pka[0:bv.shape[0], PKA_B - len(B_SLOTS) + k] = bv
    for i in range(4):
        pka[0, PKA_B + 3 * i] = inp[f"ps{i + 1}"][0]
        pka[0, PKA_B + 3 * i + 1] = inp[f"ph{i + 1}"][0]
        pka[0, PKA_B + 3 * i + 2] = inp[f"wr{i + 1}"][0]
    pkbn = np.zeros((12, 2 * L), np.float32)
    for i, k in enumerate((1, 2, 4)):
        pkbn[4 * i:4 * i + 4, 0:L] = inp[f"bn{k}_b"].reshape(4, L)
        pkbn[4 * i:4 * i + 4, L:2 * L] = inp[f"bn{k}_g"].reshape(4, L)
    return pka, pkbn


def run(inputs, trace=False, **kw):
    inputs = {k: np.ascontiguousarray(np.asarray(v, np.float32))
              for k, v in inputs.items()}
    nc = _get_program()
    pka, pkbn = _pack_params(inputs)
    xp = inputs["x"].reshape(NCORES, BPC, N, F)
    in_maps = []
    for i in range(NCORES):
        # pair-rolled full batch: core i's own pair lands in column block 0,
        # host-packed to the on-chip feature-major layout
        in_maps.append({
            "x": _pack_x(np.roll(xp, -i, axis=0).reshape(B, N, F)),
            "pka": pka,
            "pkbn": pkbn,
        })
    last_exc = None
    for attempt in range(3):
        try:
            res = run_bass_kernel_spmd(
                nc, in_maps, core_ids=list(range(NCORES)), trace=trace, **kw)
            break
        except Exception as e:  # transient NRT_EXEC_UNIT_UNRECOVERABLE flakes
            last_exc = e
            import time
            time.sleep(5)
    else:
        raise last_exc
    out = np.concatenate(
        [_unpack_out(res.results[i]["out"]) for i in range(NCORES)], axis=0)
    return out, res


def kernel(**inputs) -> np.ndarray:
    out, _ = run(inputs)
    return out
